# revision 17
# baseline (speedup 1.0000x reference)
"""Trainium2 Bass kernel for nn_CustomConv2D (degenerate conv: only the last
input channel contributes; 3x3 VALID conv -> 64 out channels + bias).

Strategy (v5 — f16 storage, memory-roofline driven):
  - The problem is HBM-traffic bound and the tolerance is 2e-2, so all HBM
    traffic is 16-bit: the im2col input is f16 (1.81 MB/core) and the
    output is computed in f32 PSUM (f16 matmul inputs, f32 bias add) but
    STORED as f16 (12.85 MB/core instead of 25.7) and upcast to f32 on the
    host. End-to-end error ~7e-4, two orders under tolerance.
  - Host builds the 9-row im2col per image, packs batch PAIRS into 18-row
    blocks (rows 0-8 img A, 9-17 img B) matching block-diagonal weights,
    so each matmul is [18 -> 128, 448] at PE quadrant offsets 0/32/64/96.
  - A single DMA's descriptors run at only ~22 GB/s (one DMA engine);
    aggregate bandwidth needs many DMAs in flight. So: the weights and
    pair 0's first segments are row-split across all three DMA-issuing
    engines (SP/Activation/GpSimd) to land fast, and every 896-col output
    chunk drains as soon as its eviction completes, alternating Sync and
    GpSimd — 10+ small DMAs stay in flight on both queues at all times,
    including the wind-down, so drains track eviction progress closely.
    Drains never issue from Scalar, whose eviction stream must not stall.
  - Compute is seg-major; 16 staging buffers hold all four pairs so the
    tensor engine never waits on staging reuse; bias is fused into the
    PSUM->SBUF evacuation (alternating VectorE / ScalarE).
"""

import sys

if "/opt/trn_rl_repo" not in sys.path:
    sys.path.insert(0, "/opt/trn_rl_repo")

import numpy as np

B, CIN, COUT, KS = 64, 64, 64, 3
H, W, HP, WP = 112, 112, 114, 114
NPIX = H * W          # 12544
NCORES = 8
BL = B // NCORES      # 8 local batches per core
PAIRS = BL // 2       # 4
KDIM = 2 * KS * KS    # 18 (9 taps x 2 images, block-diagonal weights)
NSEG = 4              # pixel segments per pair (partition offsets 0/32/64/96)
SEGW = NPIX // NSEG   # 3136
NT = 448              # pixels per matmul; 7 * 448 == 3136, fits one PSUM bank
TPS = SEGW // NT      # 7 matmul tiles per segment
HALF = 4 * NT         # half-seg drain split: cols [0, 1792) and [1792, 3136)

_CACHE = {}


def _build_bass():
    import concourse.bass as bass
    import concourse.bacc as bacc
    import concourse.mybir as mybir
    from concourse.tile import TileContext

    f32 = mybir.dt.float32
    f16 = mybir.dt.float16
    # Bacc (not plain Bass): its compile() runs move_matmul_waits_to_ldweights
    # + generate_event_semaphores, without which walrus rejects any sync wait
    # on a Matmult ("Too many sync wait commands").
    nc = bacc.Bacc("TRN2", target_bir_lowering=False, debug=False)
    mv = nc.declare_dram_parameter("mv", [PAIRS, NSEG, KDIM, SEGW], f16,
                                   isOutput=False)
    w2 = nc.declare_dram_parameter("w2", [128, 128], f16, isOutput=False)
    b2 = nc.declare_dram_parameter("b2", [128, 1], f32, isOutput=False)
    out = nc.declare_dram_parameter("out", [BL * COUT, NPIX], f16,
                                    isOutput=True)

    with TileContext(nc) as tc:
        with (
            tc.tile_pool(name="consts", bufs=1) as consts,
            tc.tile_pool(name="movp", bufs=PAIRS) as movp,
            tc.tile_pool(name="stagep", bufs=4 * PAIRS) as stagep,
            tc.tile_pool(name="psump", bufs=8, space="PSUM") as psump,
        ):
            w2_t = consts.tile([128, 128], f16)
            b2_t = consts.tile([128, 1], f32)
            movs = [movp.tile([128, SEGW + 32], f16, tag="mov",
                              name=f"mov{p}") for p in range(PAIRS)]

            def load_seg(eng, pair, s, r0, r1):
                eng.dma_start(out=movs[pair][32 * s + r0:32 * s + r1, 0:SEGW],
                              in_=mv[pair, s, r0:r1, :])

            # Issue order per engine is program order; every list below is
            # front-loaded with what gates the pipeline start. Weight rows
            # 0:64 gate segs 0-1 (which compute first), so w2 is split
            # across two engines; pair 0 seg 0 is 3-way row-split (lands
            # ~1.7us after issue vs 5us whole).
            nc.scalar.dma_start(out=w2_t[0:64, :], in_=w2[0:64, :])
            load_seg(nc.scalar, 0, 0, 0, 6)
            nc.scalar.dma_start(out=b2_t[:], in_=b2[:])
            load_seg(nc.scalar, 0, 1, 0, 9)
            # sync: w2 tail, seg0 mid, first halves of segs 1-3, then drains.
            nc.sync.dma_start(out=w2_t[64:128, :], in_=w2[64:128, :])
            load_seg(nc.sync, 0, 0, 6, 12)
            load_seg(nc.sync, 0, 2, 0, 9)
            load_seg(nc.sync, 0, 1, 9, 18)
            load_seg(nc.sync, 0, 3, 0, 9)
            # gpsimd: seg0 tail, remaining halves, pairs 1-3, then drains.
            load_seg(nc.gpsimd, 0, 0, 12, 18)
            load_seg(nc.gpsimd, 0, 2, 9, 18)
            load_seg(nc.gpsimd, 0, 3, 9, 18)
            for pair in range(1, PAIRS):
                for s in range(NSEG):
                    load_seg(nc.gpsimd, pair, s, 0, KDIM)

            def drain(eng, pair, seg, c0, c1):
                eng.dma_start(
                    out=out[pair * 128:(pair + 1) * 128,
                            seg * SEGW + c0:seg * SEGW + c1],
                    in_=stages_all[pair][seg][:, c0:c1])

            stages_all = []
            tidx = 0
            didx = 0
            for pair in range(PAIRS):
                stages = [stagep.tile([128, SEGW], f16, tag="stage",
                                      name=f"stage_{pair}_{s}")
                          for s in range(NSEG)]
                stages_all.append(stages)
                for seg in range(NSEG):
                    p0 = 32 * seg
                    for t in range(TPS):
                        n0 = t * NT
                        ps = psump.tile([128, NT], f32, tag="ps")
                        nc.tensor.matmul(ps[:, :],
                                         w2_t[p0:p0 + KDIM, :],
                                         movs[pair][p0:p0 + KDIM,
                                                    n0:n0 + NT],
                                         start=True, stop=True,
                                         tile_position=(p0, 0))
                        # PSUM -> SBUF(f16) with fused f32 bias add.
                        if tidx % 2 == 0:
                            nc.vector.tensor_scalar_add(
                                stages[seg][:, n0:n0 + NT], ps[:, :],
                                b2_t[:, :])
                        else:
                            nc.scalar.activation(
                                stages[seg][:, n0:n0 + NT], ps[:, :],
                                mybir.ActivationFunctionType.Identity,
                                bias=b2_t[:, :])
                        tidx += 1
                        # Drains: ~896-col chunks (229 KB) for every pair,
                        # alternating Sync/GpSimd issue. Small uniform chunks
                        # keep ~10+ DMAs in flight on both queues at all
                        # times — including the wind-down — so the drain
                        # stream tracks eviction progress with minimal lag.
                        # Issued only from Sync/GpSimd so ring-full stalls
                        # never block the eviction engines.
                        if t in (1, 3, 5, 6):
                            c0 = {1: 0, 3: 2 * NT, 5: 4 * NT, 6: 6 * NT}[t]
                            # GpSimd is still issuing input DMAs for the
                            # first ~8us, so the earliest drains go to Sync.
                            eng = nc.sync if (didx < 6 or didx % 2 == 0) \
                                else nc.gpsimd
                            didx += 1
                            drain(eng, pair, seg, c0, n0 + NT)
    nc.compile()
    return nc


def _get_nc():
    if "nc" not in _CACHE:
        _CACHE["nc"] = _build_bass()
    return _CACHE["nc"]


def _prep_inputs(x_padded, weight, bias):
    x = np.asarray(x_padded, dtype=np.float32)
    wt = np.asarray(weight, dtype=np.float32)
    bs = np.asarray(bias, dtype=np.float32)

    xs3 = x[:, -1, :, :]                              # [64, 114, 114]
    win = np.lib.stride_tricks.sliding_window_view(xs3, (KS, KS), axis=(1, 2))
    # [64, 112, 112, 3, 3] -> [64, 9, 12544] with row k = (i, j) shift
    mov_all = win.transpose(0, 3, 4, 1, 2).reshape(B, KS * KS, NPIX)
    # [cores, pairs, img2, 9, seg, SEGW] -> [cores, pairs, seg, (img2, 9), SEGW]
    mov_r = mov_all.reshape(NCORES, PAIRS, 2, KS * KS, NSEG, SEGW)
    mov_h = np.ascontiguousarray(
        mov_r.transpose(0, 1, 4, 2, 3, 5)
    ).reshape(NCORES, PAIRS, NSEG, KDIM, SEGW).astype(np.float16)

    wl = np.ascontiguousarray(wt[:, -1, :, :]).reshape(COUT, KS * KS)
    w2 = np.zeros((128, 128), np.float32)
    for s in range(NSEG):
        w2[32 * s: 32 * s + 9, 0:64] = wl.T
        w2[32 * s + 9: 32 * s + 18, 64:128] = wl.T
    w2 = w2.astype(np.float16)
    b2 = np.tile(bs, 2).reshape(128, 1).astype(np.float32)
    return mov_h, w2, b2


def kernel(x_padded, weight, bias, in_height=112, in_width=112, **_unused):
    from concourse.bass_utils import run_bass_kernel_spmd

    mov_h, w2, b2 = _prep_inputs(x_padded, weight, bias)
    nc = _get_nc()
    in_maps = [
        {"mv": mov_h[c], "w2": w2, "b2": b2}
        for c in range(NCORES)
    ]
    res = run_bass_kernel_spmd(nc, in_maps, core_ids=list(range(NCORES)))
    outs = [
        np.asarray(res.results[c]["out"]).astype(np.float32)
        .reshape(BL, COUT, H, W)
        for c in range(NCORES)
    ]
    return np.concatenate(outs, axis=0)


# revision 21
# speedup vs baseline: 1.0103x; 1.0103x over previous
"""Trainium2 Bass kernel for nn_CustomConv2D (degenerate conv: only the last
input channel contributes; 3x3 VALID conv -> 64 out channels + bias).

Strategy (v5 — f16 storage, memory-roofline driven):
  - The problem is HBM-traffic bound and the tolerance is 2e-2, so all HBM
    traffic is 16-bit: the im2col input is f16 (1.81 MB/core) and the
    output is computed in f32 PSUM (f16 matmul inputs, f32 bias add) but
    STORED as f16 (12.85 MB/core instead of 25.7) and upcast to f32 on the
    host. End-to-end error ~7e-4, two orders under tolerance.
  - Host builds the 9-row im2col per image, packs batch PAIRS into 18-row
    blocks (rows 0-8 img A, 9-17 img B) matching block-diagonal weights,
    so each matmul is [18 -> 128, 448] at PE quadrant offsets 0/32/64/96.
  - A single DMA's descriptors run at only ~22 GB/s (one DMA engine);
    aggregate bandwidth needs many DMAs in flight. So: pair 0's first
    segments are row-split across all three DMA-issuing engines
    (SP/Activation/GpSimd) to land fast, pair 0 and pair 3 drain in
    ~896-col chunks (high concurrency during ramp-up and wind-down), pairs
    1-2 drain in half-segments. Drains are issued only from Sync/GpSimd so
    ring-full stalls never block the Scalar eviction stream.
  - Compute is seg-major; 16 staging buffers hold all four pairs so the
    tensor engine never waits on staging reuse; bias is fused into the
    PSUM->SBUF evacuation (alternating VectorE / ScalarE).
"""

import sys

if "/opt/trn_rl_repo" not in sys.path:
    sys.path.insert(0, "/opt/trn_rl_repo")

import numpy as np

B, CIN, COUT, KS = 64, 64, 64, 3
H, W, HP, WP = 112, 112, 114, 114
NPIX = H * W          # 12544
NCORES = 8
BL = B // NCORES      # 8 local batches per core
PAIRS = BL // 2       # 4
KDIM = 2 * KS * KS    # 18 (9 taps x 2 images, block-diagonal weights)
NSEG = 4              # pixel segments per pair (partition offsets 0/32/64/96)
SEGW = NPIX // NSEG   # 3136
NT = 448              # pixels per matmul; 7 * 448 == 3136, fits one PSUM bank
TPS = SEGW // NT      # 7 matmul tiles per segment
HALF = 4 * NT         # half-seg drain split: cols [0, 1792) and [1792, 3136)

_CACHE = {}


def _build_bass():
    import concourse.bass as bass
    import concourse.bacc as bacc
    import concourse.mybir as mybir
    from concourse.tile import TileContext

    f32 = mybir.dt.float32
    f16 = mybir.dt.float16
    # Bacc (not plain Bass): its compile() runs move_matmul_waits_to_ldweights
    # + generate_event_semaphores, without which walrus rejects any sync wait
    # on a Matmult ("Too many sync wait commands").
    nc = bacc.Bacc("TRN2", target_bir_lowering=False, debug=False)
    mv = nc.declare_dram_parameter("mv", [PAIRS, NSEG, KDIM, SEGW], f16,
                                   isOutput=False)
    w2 = nc.declare_dram_parameter("w2", [128, 128], f16, isOutput=False)
    b2 = nc.declare_dram_parameter("b2", [128, 1], f32, isOutput=False)
    out = nc.declare_dram_parameter("out", [BL * COUT, NPIX], f16,
                                    isOutput=True)

    with TileContext(nc) as tc:
        with (
            tc.tile_pool(name="consts", bufs=1) as consts,
            tc.tile_pool(name="movp", bufs=PAIRS) as movp,
            tc.tile_pool(name="stagep", bufs=4 * PAIRS) as stagep,
            tc.tile_pool(name="psump", bufs=8, space="PSUM") as psump,
        ):
            w2_t = consts.tile([128, 128], f16)
            b2_t = consts.tile([128, 1], f32)
            movs = [movp.tile([128, SEGW + 32], f16, tag="mov",
                              name=f"mov{p}") for p in range(PAIRS)]

            def load_seg(eng, pair, s, r0, r1):
                eng.dma_start(out=movs[pair][32 * s + r0:32 * s + r1, 0:SEGW],
                              in_=mv[pair, s, r0:r1, :])

            # Issue order per engine is program order; every list below is
            # front-loaded with what gates the pipeline start. Weight rows
            # 0:64 gate segs 0-1 (which compute first), so w2 is split
            # across two engines; pair 0 seg 0 is 3-way row-split (lands
            # ~1.7us after issue vs 5us whole).
            nc.scalar.dma_start(out=w2_t[0:64, :], in_=w2[0:64, :])
            load_seg(nc.scalar, 0, 0, 0, 6)
            nc.scalar.dma_start(out=b2_t[:], in_=b2[:])
            load_seg(nc.scalar, 0, 1, 0, 9)
            # sync: w2 tail, seg0 mid, first halves of segs 1-3, then drains.
            nc.sync.dma_start(out=w2_t[64:128, :], in_=w2[64:128, :])
            load_seg(nc.sync, 0, 0, 6, 12)
            load_seg(nc.sync, 0, 2, 0, 9)
            load_seg(nc.sync, 0, 1, 9, 18)
            load_seg(nc.sync, 0, 3, 0, 9)
            # gpsimd: seg0 tail, remaining halves, pairs 1-3, then drains.
            load_seg(nc.gpsimd, 0, 0, 12, 18)
            load_seg(nc.gpsimd, 0, 2, 9, 18)
            load_seg(nc.gpsimd, 0, 3, 9, 18)
            for pair in range(1, PAIRS):
                for s in range(NSEG):
                    load_seg(nc.gpsimd, pair, s, 0, KDIM)

            def drain(eng, pair, seg, c0, c1, r0=0, r1=128):
                eng.dma_start(
                    out=out[pair * 128 + r0:pair * 128 + r1,
                            seg * SEGW + c0:seg * SEGW + c1],
                    in_=stages_all[pair][seg][r0:r1, c0:c1])

            stages_all = []
            tidx = 0
            didx = 0
            for pair in range(PAIRS):
                stages = [stagep.tile([128, SEGW], f16, tag="stage",
                                      name=f"stage_{pair}_{s}")
                          for s in range(NSEG)]
                stages_all.append(stages)
                for seg in range(NSEG):
                    p0 = 32 * seg
                    for t in range(TPS):
                        n0 = t * NT
                        ps = psump.tile([128, NT], f32, tag="ps")
                        nc.tensor.matmul(ps[:, :],
                                         w2_t[p0:p0 + KDIM, :],
                                         movs[pair][p0:p0 + KDIM,
                                                    n0:n0 + NT],
                                         start=True, stop=True,
                                         tile_position=(p0, 0))
                        # PSUM -> SBUF(f16) with fused f32 bias add.
                        if tidx % 2 == 0:
                            nc.vector.tensor_scalar_add(
                                stages[seg][:, n0:n0 + NT], ps[:, :],
                                b2_t[:, :])
                        else:
                            nc.scalar.activation(
                                stages[seg][:, n0:n0 + NT], ps[:, :],
                                mybir.ActivationFunctionType.Identity,
                                bias=b2_t[:, :])
                        tidx += 1
                        # Drains (issued from Sync/GpSimd only, so ring-full
                        # stalls never block the Scalar eviction stream).
                        # Pair 0: ~896-col chunks for an early stream start.
                        # Pairs 1-2: half-segs — 3584-byte descriptors give
                        # full per-DMA bandwidth and the deep queue backlog
                        # keeps concurrency high mid-run. Pair 3: half-segs
                        # split by PARTITION rows — small drains that keep
                        # fat descriptors, so the wind-down stays at high
                        # concurrency AND full per-descriptor bandwidth.
                        if pair == 0:
                            if t in (1, 3, 5, 6):
                                c0 = {1: 0, 3: 2 * NT, 5: 4 * NT,
                                      6: 6 * NT}[t]
                                drain(nc.sync, pair, seg, c0, n0 + NT)
                        elif t == 3 or t == TPS - 1:
                            c0 = 0 if t == 3 else HALF
                            c1 = HALF if t == 3 else SEGW
                            if pair == PAIRS - 1:
                                drain(nc.sync if didx % 2 == 0 else
                                      nc.gpsimd, pair, seg, c0, c1, 0, 64)
                                drain(nc.gpsimd if didx % 2 == 0 else
                                      nc.sync, pair, seg, c0, c1, 64, 128)
                            else:
                                eng = nc.sync if didx % 2 == 0 else nc.gpsimd
                                drain(eng, pair, seg, c0, c1)
                            didx += 1
    nc.compile()
    return nc


def _get_nc():
    if "nc" not in _CACHE:
        _CACHE["nc"] = _build_bass()
    return _CACHE["nc"]


def _prep_inputs(x_padded, weight, bias):
    x = np.asarray(x_padded, dtype=np.float32)
    wt = np.asarray(weight, dtype=np.float32)
    bs = np.asarray(bias, dtype=np.float32)

    xs3 = x[:, -1, :, :]                              # [64, 114, 114]
    win = np.lib.stride_tricks.sliding_window_view(xs3, (KS, KS), axis=(1, 2))
    # [64, 112, 112, 3, 3] -> [64, 9, 12544] with row k = (i, j) shift
    mov_all = win.transpose(0, 3, 4, 1, 2).reshape(B, KS * KS, NPIX)
    # [cores, pairs, img2, 9, seg, SEGW] -> [cores, pairs, seg, (img2, 9), SEGW]
    mov_r = mov_all.reshape(NCORES, PAIRS, 2, KS * KS, NSEG, SEGW)
    mov_h = np.ascontiguousarray(
        mov_r.transpose(0, 1, 4, 2, 3, 5)
    ).reshape(NCORES, PAIRS, NSEG, KDIM, SEGW).astype(np.float16)

    wl = np.ascontiguousarray(wt[:, -1, :, :]).reshape(COUT, KS * KS)
    w2 = np.zeros((128, 128), np.float32)
    for s in range(NSEG):
        w2[32 * s: 32 * s + 9, 0:64] = wl.T
        w2[32 * s + 9: 32 * s + 18, 64:128] = wl.T
    w2 = w2.astype(np.float16)
    b2 = np.tile(bs, 2).reshape(128, 1).astype(np.float32)
    return mov_h, w2, b2


def kernel(x_padded, weight, bias, in_height=112, in_width=112, **_unused):
    from concourse.bass_utils import run_bass_kernel_spmd

    mov_h, w2, b2 = _prep_inputs(x_padded, weight, bias)
    nc = _get_nc()
    in_maps = [
        {"mv": mov_h[c], "w2": w2, "b2": b2}
        for c in range(NCORES)
    ]
    res = run_bass_kernel_spmd(nc, in_maps, core_ids=list(range(NCORES)))
    outs = [
        np.asarray(res.results[c]["out"]).astype(np.float32)
        .reshape(BL, COUT, H, W)
        for c in range(NCORES)
    ]
    return np.concatenate(outs, axis=0)


# revision 22
# speedup vs baseline: 1.0427x; 1.0321x over previous
"""Trainium2 Bass kernel for nn_CustomConv2D (degenerate conv: only the last
input channel contributes; 3x3 VALID conv -> 64 out channels + bias).

Strategy (v5 — f16 storage, memory-roofline driven):
  - The problem is HBM-traffic bound and the tolerance is 2e-2, so all HBM
    traffic is 16-bit: the im2col input is f16 (1.81 MB/core) and the
    output is computed in f32 PSUM (f16 matmul inputs, f32 bias add) but
    STORED as f16 (12.85 MB/core instead of 25.7) and upcast to f32 on the
    host. End-to-end error ~7e-4, two orders under tolerance.
  - Host builds the 9-row im2col per image, packs batch PAIRS into 18-row
    blocks (rows 0-8 img A, 9-17 img B) matching block-diagonal weights,
    so each matmul is [18 -> 128, 448] at PE quadrant offsets 0/32/64/96.
  - A single DMA's descriptors run at only ~22 GB/s (one DMA engine);
    aggregate bandwidth needs many DMAs in flight. So: pair 0's first
    segments are row-split across all three DMA-issuing engines
    (SP/Activation/GpSimd) to land fast, pair 0 and pair 3 drain in
    ~896-col chunks (high concurrency during ramp-up and wind-down), pairs
    1-2 drain in half-segments. Drains are issued only from Sync/GpSimd so
    ring-full stalls never block the Scalar eviction stream.
  - Compute is seg-major; 16 staging buffers hold all four pairs so the
    tensor engine never waits on staging reuse; bias is fused into the
    PSUM->SBUF evacuation (alternating VectorE / ScalarE).
"""

import sys

if "/opt/trn_rl_repo" not in sys.path:
    sys.path.insert(0, "/opt/trn_rl_repo")

import numpy as np

B, CIN, COUT, KS = 64, 64, 64, 3
H, W, HP, WP = 112, 112, 114, 114
NPIX = H * W          # 12544
NCORES = 8
BL = B // NCORES      # 8 local batches per core
PAIRS = BL // 2       # 4
KDIM = 2 * KS * KS    # 18 (9 taps x 2 images, block-diagonal weights)
NSEG = 4              # pixel segments per pair (partition offsets 0/32/64/96)
SEGW = NPIX // NSEG   # 3136
NT = 448              # pixels per matmul; 7 * 448 == 3136, fits one PSUM bank
TPS = SEGW // NT      # 7 matmul tiles per segment
HALF = 4 * NT         # half-seg drain split: cols [0, 1792) and [1792, 3136)

_CACHE = {}


def _build_bass():
    import concourse.bass as bass
    import concourse.bacc as bacc
    import concourse.mybir as mybir
    from concourse.tile import TileContext

    f32 = mybir.dt.float32
    f16 = mybir.dt.float16
    # Bacc (not plain Bass): its compile() runs move_matmul_waits_to_ldweights
    # + generate_event_semaphores, without which walrus rejects any sync wait
    # on a Matmult ("Too many sync wait commands").
    nc = bacc.Bacc("TRN2", target_bir_lowering=False, debug=False)
    mv = nc.declare_dram_parameter("mv", [PAIRS, NSEG, KDIM, SEGW], f16,
                                   isOutput=False)
    w2 = nc.declare_dram_parameter("w2", [128, 128], f16, isOutput=False)
    b2 = nc.declare_dram_parameter("b2", [128, 1], f32, isOutput=False)
    out = nc.declare_dram_parameter("out", [BL * COUT, NPIX], f16,
                                    isOutput=True)

    with TileContext(nc) as tc:
        with (
            tc.tile_pool(name="consts", bufs=1) as consts,
            tc.tile_pool(name="movp", bufs=PAIRS) as movp,
            tc.tile_pool(name="stagep", bufs=4 * PAIRS) as stagep,
            tc.tile_pool(name="psump", bufs=8, space="PSUM") as psump,
        ):
            w2_t = consts.tile([128, 128], f16)
            b2_t = consts.tile([128, 1], f32)
            movs = [movp.tile([128, SEGW + 32], f16, tag="mov",
                              name=f"mov{p}") for p in range(PAIRS)]

            def load_seg(eng, pair, s, r0, r1):
                eng.dma_start(out=movs[pair][32 * s + r0:32 * s + r1, 0:SEGW],
                              in_=mv[pair, s, r0:r1, :])

            # Issue order per engine is program order; every list below is
            # front-loaded with what gates the pipeline start. Pair 0 seg 0
            # is 3-way row-split (lands ~1.7us after issue vs 5us whole).
            # scalar: seg0 part, weights, bias, seg1 part, then evictions.
            load_seg(nc.scalar, 0, 0, 0, 6)
            nc.scalar.dma_start(out=w2_t[:], in_=w2[:])
            nc.scalar.dma_start(out=b2_t[:], in_=b2[:])
            load_seg(nc.scalar, 0, 1, 0, 9)
            # sync: seg0 + first halves of segs 1-3, then drains.
            load_seg(nc.sync, 0, 0, 6, 12)
            load_seg(nc.sync, 0, 2, 0, 9)
            load_seg(nc.sync, 0, 1, 9, 18)
            load_seg(nc.sync, 0, 3, 0, 9)
            # gpsimd: seg0 tail, remaining halves, pairs 1-3, then drains.
            load_seg(nc.gpsimd, 0, 0, 12, 18)
            load_seg(nc.gpsimd, 0, 2, 9, 18)
            load_seg(nc.gpsimd, 0, 3, 9, 18)
            for pair in range(1, PAIRS):
                for s in range(NSEG):
                    load_seg(nc.gpsimd, pair, s, 0, KDIM)

            def drain(eng, pair, seg, c0, c1):
                eng.dma_start(
                    out=out[pair * 128:(pair + 1) * 128,
                            seg * SEGW + c0:seg * SEGW + c1],
                    in_=stages_all[pair][seg][:, c0:c1])

            stages_all = []
            tidx = 0
            didx = 0
            for pair in range(PAIRS):
                stages = [stagep.tile([128, SEGW], f16, tag="stage",
                                      name=f"stage_{pair}_{s}")
                          for s in range(NSEG)]
                stages_all.append(stages)
                for seg in range(NSEG):
                    p0 = 32 * seg
                    for t in range(TPS):
                        n0 = t * NT
                        ps = psump.tile([128, NT], f32, tag="ps")
                        nc.tensor.matmul(ps[:, :],
                                         w2_t[p0:p0 + KDIM, :],
                                         movs[pair][p0:p0 + KDIM,
                                                    n0:n0 + NT],
                                         start=True, stop=True,
                                         tile_position=(p0, 0))
                        # PSUM -> SBUF(f16) with fused f32 bias add.
                        if tidx % 2 == 0:
                            nc.vector.tensor_scalar_add(
                                stages[seg][:, n0:n0 + NT], ps[:, :],
                                b2_t[:, :])
                        else:
                            nc.scalar.activation(
                                stages[seg][:, n0:n0 + NT], ps[:, :],
                                mybir.ActivationFunctionType.Identity,
                                bias=b2_t[:, :])
                        tidx += 1
                        # Drains (issued from Sync/GpSimd only, so ring-full
                        # stalls never block the Scalar eviction stream).
                        # Pairs 0 and 3: ~896-col chunks for high concurrency
                        # while the stream ramps up / winds down. Pairs 1-2:
                        # half-segs (fewer issue slots; the deep queue
                        # backlog keeps concurrency high mid-run).
                        if pair == 0 or pair == PAIRS - 1:
                            if t in (1, 3, 5, 6):
                                c0 = {1: 0, 3: 2 * NT, 5: 4 * NT,
                                      6: 6 * NT}[t]
                                eng = nc.sync if (pair == 0 or
                                                  didx % 2 == 0) \
                                    else nc.gpsimd
                                didx += pair != 0
                                drain(eng, pair, seg, c0, n0 + NT)
                        elif t == 3 or t == TPS - 1:
                            c0 = 0 if t == 3 else HALF
                            c1 = HALF if t == 3 else SEGW
                            eng = nc.sync if didx % 2 == 0 else nc.gpsimd
                            didx += 1
                            drain(eng, pair, seg, c0, c1)
    nc.compile()
    return nc


def _get_nc():
    if "nc" not in _CACHE:
        _CACHE["nc"] = _build_bass()
    return _CACHE["nc"]


def _prep_inputs(x_padded, weight, bias):
    x = np.asarray(x_padded, dtype=np.float32)
    wt = np.asarray(weight, dtype=np.float32)
    bs = np.asarray(bias, dtype=np.float32)

    xs3 = x[:, -1, :, :]                              # [64, 114, 114]
    win = np.lib.stride_tricks.sliding_window_view(xs3, (KS, KS), axis=(1, 2))
    # [64, 112, 112, 3, 3] -> [64, 9, 12544] with row k = (i, j) shift
    mov_all = win.transpose(0, 3, 4, 1, 2).reshape(B, KS * KS, NPIX)
    # [cores, pairs, img2, 9, seg, SEGW] -> [cores, pairs, seg, (img2, 9), SEGW]
    mov_r = mov_all.reshape(NCORES, PAIRS, 2, KS * KS, NSEG, SEGW)
    mov_h = np.ascontiguousarray(
        mov_r.transpose(0, 1, 4, 2, 3, 5)
    ).reshape(NCORES, PAIRS, NSEG, KDIM, SEGW).astype(np.float16)

    wl = np.ascontiguousarray(wt[:, -1, :, :]).reshape(COUT, KS * KS)
    w2 = np.zeros((128, 128), np.float32)
    for s in range(NSEG):
        w2[32 * s: 32 * s + 9, 0:64] = wl.T
        w2[32 * s + 9: 32 * s + 18, 64:128] = wl.T
    w2 = w2.astype(np.float16)
    b2 = np.tile(bs, 2).reshape(128, 1).astype(np.float32)
    return mov_h, w2, b2


def kernel(x_padded, weight, bias, in_height=112, in_width=112, **_unused):
    from concourse.bass_utils import run_bass_kernel_spmd

    mov_h, w2, b2 = _prep_inputs(x_padded, weight, bias)
    nc = _get_nc()
    in_maps = [
        {"mv": mov_h[c], "w2": w2, "b2": b2}
        for c in range(NCORES)
    ]
    res = run_bass_kernel_spmd(nc, in_maps, core_ids=list(range(NCORES)))
    outs = [
        np.asarray(res.results[c]["out"]).astype(np.float32)
        .reshape(BL, COUT, H, W)
        for c in range(NCORES)
    ]
    return np.concatenate(outs, axis=0)
